# revision 24
# baseline (speedup 1.0000x reference)
"""Tensor-parallel MoE MLP (Llama4 text experts) for 8 Trainium2 NeuronCores.

Strategy: 8-way tensor parallel over the intermediate dim F. Core s holds a
256-column slice of every expert's gate/up projection and the matching
256-row slice of its down projection (12 MB bf16 total, SBUF-resident).
Every core processes ALL 8192 tokens (sorted by expert id, chunked <=512
tokens per PSUM tile, chunk boundaries aligned to expert boundaries):
    h_s = up_s * silu(gate_s),   y_s = h_s @ Wd_s      (per-core partial)
    y   = sum_s y_s                                    (host-side add, free)
Compute per core is exactly total/8 independent of the expert distribution
(vs expert-parallel, which pays for the largest expert). Computed transposed
(weights stationary, tokens streaming); bf16 inputs, fp32 PSUM, bf16 partial
outputs (the host accumulates in fp32; adds ~0.4% rel err vs the 2e-2 gate).

Perf notes:
  - PE floor is 8192 tok x 48 cyc = 393216 cyc = 163.8us; expert-parallel's
    floor is max_count x 384 cyc (~173.4us for the graded seed).
  - First chunk is deliberately small (256 tokens) so the critical DMA set
    (pair-0 weights + half the x chunk, ~0.75 MB) lands early; warmup
    matmuls on zeroed SBUF lift the PE HAM clock gate meanwhile.
  - x chunks / weight pieces are released in consumption order, a few in
    flight at a time (GpSimd copy links gated on compute progress), so the
    SDMA packet round-robin cannot starve the critical transfers.
  - y partials stream out per chunk on the GpSimd (SWDGE) queue; the last
    chunk is small (128 tokens) and its output is split to shorten the tail.
"""

import numpy as np
import ml_dtypes

_BF16 = ml_dtypes.bfloat16
_NC = 8  # cores

_nc_cache: dict = {}
last_run = None  # BassKernelResults of the most recent kernel() call


def _equal_split(n, cap=512):
    if n <= 0:
        return []
    k = -(-n // cap)
    q, r = divmod(n, k)
    return [q + 1 if j < r else q for j in range(k)]


def _build(chunk_ns, chunk_epos, n_experts):
    import concourse.bacc as bacc
    import concourse.mybir as mybir
    from concourse.tile import TileContext

    nc = bacc.Bacc()
    bf16 = mybir.dt.bfloat16
    f32 = mybir.dt.float32

    KB1 = 8                      # H/128 contraction chunks for gate_up
    NCH = len(chunk_ns)
    n0 = chunk_ns[0]
    # column offset of each chunk in the packed x / y layouts
    xoff = [0]
    for n in chunk_ns:
        xoff.append(xoff[-1] + n)

    # DRAM inputs (host-packed, every DMA a whole-tensor contiguous copy)
    xc0a = nc.dram_tensor("xc0a", [128, 4 * n0], bf16, kind="ExternalInput")
    xc0b = nc.dram_tensor("xc0b", [128, 4 * n0], bf16, kind="ExternalInput")
    xcs = {
        i: nc.dram_tensor(f"xc{i}", [128, 8 * chunk_ns[i]], bf16, kind="ExternalInput")
        for i in range(1, NCH)
    }
    # expert 0's pair-0 weights split gate/up so the first-matmul critical
    # DMA is the 256 KB gate piece alone
    wgu0g = nc.dram_tensor("wgu0g", [128, 1024], bf16, kind="ExternalInput")
    wgu0u = nc.dram_tensor("wgu0u", [128, 1024], bf16, kind="ExternalInput")
    wgu0b = nc.dram_tensor("wgu0b", [128, 2048], bf16, kind="ExternalInput")
    wgus = {
        p: nc.dram_tensor(f"wgu{p}", [128, 4096], bf16, kind="ExternalInput")
        for p in range(1, n_experts)
    }
    n_wdn = -(-n_experts // 2)
    wdns = {
        q: nc.dram_tensor(
            f"wdn{q}", [128, 2048 * min(2, n_experts - 2 * q)], bf16,
            kind="ExternalInput",
        )
        for q in range(n_wdn)
    }
    yT = nc.dram_tensor("yT", [128, 8 * xoff[-1]], bf16, kind="ExternalOutput")

    # first chunk index of each expert position (for weight prefetch pacing)
    first_chunk_of_pos = {}
    for i, p in enumerate(chunk_epos):
        first_chunk_of_pos.setdefault(p, i)

    with TileContext(nc) as tc:
        with (
            tc.tile_pool(name="wpool", bufs=1) as wpool,
            tc.tile_pool(name="xpool", bufs=7) as xpool,
            tc.tile_pool(name="warm", bufs=1) as warm_p,
            tc.tile_pool(name="silu_p", bufs=3) as silu_p,
            tc.tile_pool(name="h_p", bufs=2) as h_p,
            tc.tile_pool(name="y_p", bufs=5) as y_p,
            tc.tile_pool(name="ps1", bufs=3, space="PSUM") as ps1_p,
            tc.tile_pool(name="ps2", bufs=5, space="PSUM") as ps2_p,
        ):
            # PE warm-up: dummy matmuls on zeroed SBUF while the critical
            # inputs stream in, lifting the HAM clock gate (1.2 -> 2.4 GHz)
            # before the real matmul stream.
            wtile = warm_p.tile([128, 256], bf16, name="wtile", tag="wtile")
            nc.vector.memset(wtile[:], 0.0)
            for i in range(60):
                ps_w = ps2_p.tile([128, 512], f32, tag="ps2")
                nc.tensor.matmul(
                    out=ps_w[:, :128],
                    lhsT=wtile[:, 128:256],
                    rhs=wtile[:, 0:128],
                    start=True,
                    stop=True,
                )

            # Resident weight tiles
            wgu_sb = wpool.tile([128, n_experts * 4096], bf16, name="wgu", tag="wgu")
            wdn_sb = wpool.tile([128, n_experts * 2048], bf16, name="wdn", tag="wdn")

            # Critical DMAs via SWDGE, strictly in first-use order: gate
            # pair-0 weights, then x chunk 0 (both halves), then up pair-0.
            # Nothing else competes for SDMA packets until these land.
            xt = {0: xpool.tile([128, 4096], bf16, name="xt0", tag="x")}
            nc.gpsimd.dma_start(out=wgu_sb[:, 0:1024], in_=wgu0g[:, :])
            nc.gpsimd.dma_start(out=xt[0][:, : 4 * n0], in_=xc0a[:, :])
            nc.gpsimd.dma_start(out=xt[0][:, 4 * n0 : 8 * n0], in_=xc0b[:, :])
            # pair-0 up / pair-1 weights on the sync ring in parallel with
            # the gpsimd critical set (the two rings share SDMA bandwidth
            # round-robin, so everything early streams concurrently)
            nc.sync.dma_start(out=wgu_sb[:, 1024:2048], in_=wgu0u[:, :])
            nc.sync.dma_start(out=wgu_sb[:, 2048:4096], in_=wgu0b[:, :])

            def release(dst_head, src_ap, engine, dst_ap, src_dram):
                # Gate a bulk DMA on earlier data/compute: GpSimd copy into
                # the head of the DMA's dest region, then the dma_start has
                # a WAW dependency on that copy.
                nc.gpsimd.tensor_copy(dst_head, src_ap)
                engine.dma_start(out=dst_ap, in_=src_dram)

            # chained early releases, gated on the first critical pieces
            wdn0_w = 2048 * min(2, n_experts)
            xc0a_tail = xt[0][:, 4 * n0 - 4 : 4 * n0]
            if NCH > 1:
                n1 = chunk_ns[1]
                xt[1] = xpool.tile([128, 4096], bf16, name="xt1", tag="x")
                release(
                    xt[1][:, :4], wgu_sb[:, 1020:1024],
                    nc.sync, xt[1][:, : 8 * n1], xcs[1][:, :],
                )
            release(
                wdn_sb[:, :4], xc0a_tail,
                nc.scalar, wdn_sb[:, :wdn0_w], wdns[0][:, :],
            )

            h_tiles = {}
            released_w = {0}
            released_dn = {0}

            def prefetch(i):
                # During chunk i's emission: release x chunk i+2 and any
                # weight pieces needed within the next ~2 chunks, gated on
                # chunk i-1's h tile (i.e. on compute progress).
                if i < 1:
                    return
                gate_src = h_tiles[(i - 1, 1)][:, :4]
                for j in range(i + 1, min(i + 4, NCH)):
                    if j not in xt:
                        nj = chunk_ns[j]
                        xt[j] = xpool.tile([128, 4096], bf16, name=f"xt{j}", tag="x")
                        release(
                            xt[j][:, :4], gate_src,
                            nc.sync, xt[j][:, : 8 * nj], xcs[j][:, :],
                        )
                # weight pieces for experts starting within ~5 chunks
                nw = 0
                for p in range(1, n_experts):
                    if p in released_w:
                        continue
                    if first_chunk_of_pos[p] <= i + 5:
                        release(
                            wgu_sb[:, p * 4096 : p * 4096 + 4], gate_src,
                            nc.scalar,
                            wgu_sb[:, p * 4096 : (p + 1) * 4096],
                            wgus[p][:, :],
                        )
                        released_w.add(p)
                        nw += 1
                        q = p // 2
                        if q not in released_dn:
                            o = q * 4096
                            w_q = 2048 * min(2, n_experts - 2 * q)
                            release(
                                wdn_sb[:, o : o + 4], gate_src,
                                nc.scalar,
                                wdn_sb[:, o : o + w_q],
                                wdns[q][:, :],
                            )
                            released_dn.add(q)
                        if nw >= 2:
                            break


            def gu_pair(i, p):
                # weight layout per (expert, pair): [gate k0..7 | up k0..7]
                e, n = chunk_epos[i], chunk_ns[i]
                ps_g = ps1_p.tile([128, 512], f32, tag="ps1")
                for k in range(KB1):
                    o = e * 4096 + p * 2048 + k * 128
                    nc.tensor.matmul(
                        out=ps_g[:, :n],
                        lhsT=wgu_sb[:, o : o + 128],
                        rhs=xt[i][:, k * n : k * n + n],
                        start=(k == 0),
                        stop=(k == KB1 - 1),
                    )
                ps_u = ps1_p.tile([128, 512], f32, tag="ps1")
                for k in range(KB1):
                    o = e * 4096 + p * 2048 + 1024 + k * 128
                    nc.tensor.matmul(
                        out=ps_u[:, :n],
                        lhsT=wgu_sb[:, o : o + 128],
                        rhs=xt[i][:, k * n : k * n + n],
                        start=(k == 0),
                        stop=(k == KB1 - 1),
                    )
                st = silu_p.tile([128, 512], bf16, tag="silu")
                nc.scalar.activation(
                    st[:, :n], ps_g[:, :n], mybir.ActivationFunctionType.Silu
                )
                ht = h_p.tile([128, 512], bf16, tag=f"h{p}")
                nc.vector.tensor_mul(out=ht[:, :n], in0=ps_u[:, :n], in1=st[:, :n])
                h_tiles[(i, p)] = ht

            def dn(i):
                e, n = chunk_epos[i], chunk_ns[i]
                last = i == NCH - 1
                yst = y_p.tile([128, 4096], bf16, tag="y")
                oc = 8 * xoff[i]
                for hh in range(8):
                    halves = [(0, n)]
                    if last and hh == 7:
                        halves = [(0, n // 2), (n // 2, n - n // 2)]
                    for c0, cn in halves:
                        ps_y = ps2_p.tile([128, 512], f32, tag="ps2")
                        for k2 in range(2):
                            nc.tensor.matmul(
                                out=ps_y[:, :cn],
                                lhsT=wdn_sb[:, e * 2048 + k2 * 1024 + hh * 128 : e * 2048 + k2 * 1024 + hh * 128 + 128],
                                rhs=h_tiles[(i, k2)][:, c0 : c0 + cn],
                                start=(k2 == 0),
                                stop=(k2 == 1),
                            )
                        # alternate PSUM->SBUF drains between DVE and ACT so
                        # neither engine's backlog stalls the dn matmul groups
                        dst = yst[:, hh * n + c0 : hh * n + c0 + cn]
                        if hh % 2 == 0:
                            nc.vector.tensor_copy(dst, ps_y[:, :cn])
                        else:
                            nc.scalar.activation(
                                dst, ps_y[:, :cn], mybir.ActivationFunctionType.Copy
                            )
                    if last and hh == 3:
                        nc.scalar.dma_start(
                            out=yT[:, oc : oc + 4 * n], in_=yst[:, : 4 * n]
                        )
                    if last and hh == 5:
                        nc.sync.dma_start(
                            out=yT[:, oc + 4 * n : oc + 6 * n],
                            in_=yst[:, 4 * n : 6 * n],
                        )
                    if last and hh == 7:
                        nc.scalar.dma_start(
                            out=yT[:, oc + 6 * n : oc + 7 * n + n // 2],
                            in_=yst[:, 6 * n : 7 * n + n // 2],
                        )
                del h_tiles[(i, 0)], h_tiles[(i, 1)]
                if last:
                    nc.sync.dma_start(
                        out=yT[:, oc + 7 * n + n // 2 : oc + 8 * n],
                        in_=yst[:, 7 * n + n // 2 : 8 * n],
                    )
                elif i >= NCH - 4:
                    # near the end, use the (now idle) sync HWDGE ring so
                    # output completion receipts don't delay the teardown
                    nc.sync.dma_start(out=yT[:, oc : oc + 8 * n], in_=yst[:, : 8 * n])
                else:
                    nc.gpsimd.dma_start(out=yT[:, oc : oc + 8 * n], in_=yst[:, : 8 * n])

            # Pipeline order: dn(i-1) is sandwiched BETWEEN chunk i's two
            # gu pairs. The PE bridge (gu pair) covers the silu+mul latency
            # of h(i-1), and the dn PSUM drains land in DVE/ACT FIFO order
            # right as the dn groups complete (no ps2 bank WAR exposure).
            for i in range(NCH):
                prefetch(i)
                gu_pair(i, 0)
                if i >= 1:
                    dn(i - 1)
                gu_pair(i, 1)
            dn(NCH - 1)
    nc.compile()
    return nc


def kernel(hidden_states, local_expert_indices, gate_up_proj, down_proj):
    from concourse.bass_utils import run_bass_kernel_spmd

    x = np.asarray(hidden_states, dtype=np.float32)
    idx = np.asarray(local_expert_indices).astype(np.int64)
    wgu_all = np.asarray(gate_up_proj, dtype=np.float32)
    wd_all = np.asarray(down_proj, dtype=np.float32)

    T, H = x.shape
    E, _, F2 = wgu_all.shape
    F = F2 // 2

    order = np.argsort(idx, kind="stable")
    counts = np.bincount(idx, minlength=E)
    experts = [e for e in range(E) if counts[e] > 0]
    n_experts = len(experts)

    # chunk plan: per expert, <=512-token chunks; small first chunk so the
    # critical first DMAs are small (fast ramp)
    chunk_ns, chunk_epos = [], []
    for pos, e in enumerate(experts):
        n = int(counts[e])
        if pos == 0 and n >= 320:
            sizes = [256] + _equal_split(n - 256)
        else:
            sizes = _equal_split(n)
        chunk_ns += sizes
        chunk_epos += [pos] * len(sizes)
    chunk_ns = tuple(chunk_ns)
    chunk_epos = tuple(chunk_epos)

    key = (H, F, chunk_ns, chunk_epos, n_experts)
    if key not in _nc_cache:
        _nc_cache[key] = _build(chunk_ns, chunk_epos, n_experts)
    nc = _nc_cache[key]

    # ---- pack shared x chunks (identical for every core) ----
    x_sorted = x[order].astype(_BF16)
    KB1 = H // 128
    shared = {}
    pos_tok = 0
    for i, n in enumerate(chunk_ns):
        blk = np.ascontiguousarray(
            x_sorted[pos_tok : pos_tok + n].reshape(n, KB1, 128).transpose(2, 1, 0)
        ).reshape(128, KB1 * n)
        if i == 0:
            shared["xc0a"] = np.ascontiguousarray(blk[:, : 4 * n])
            shared["xc0b"] = np.ascontiguousarray(blk[:, 4 * n :])
        else:
            shared[f"xc{i}"] = blk
        pos_tok += n

    # ---- per-core weight slices ----
    in_maps = []
    for s in range(_NC):
        sl = slice(s * 256, (s + 1) * 256)
        m = dict(shared)
        dn_pieces = {}
        for p, e in enumerate(experts):
            w = wgu_all[e]
            g = w[:, :F][:, sl].astype(_BF16)
            u = w[:, F:][:, sl].astype(_BF16)

            def kmaj(blk128):  # [H, 128] -> [128, KB1*128] (k-major lhsT)
                return np.ascontiguousarray(
                    blk128.reshape(KB1, 128, 128).transpose(1, 0, 2)
                ).reshape(128, KB1 * 128)

            # per pair: [gate k-major (1024) | up k-major (1024)]
            pair_blocks = [
                np.concatenate(
                    [kmaj(g[:, pp * 128 : (pp + 1) * 128]),
                     kmaj(u[:, pp * 128 : (pp + 1) * 128])],
                    axis=1,
                )
                for pp in range(2)
            ]
            if p == 0:
                m["wgu0g"] = np.ascontiguousarray(pair_blocks[0][:, :1024])
                m["wgu0u"] = np.ascontiguousarray(pair_blocks[0][:, 1024:])
                m["wgu0b"] = np.ascontiguousarray(pair_blocks[1])
            else:
                m[f"wgu{p}"] = np.ascontiguousarray(
                    np.concatenate(pair_blocks, axis=1)
                )
            d = wd_all[e][sl, :].astype(_BF16)  # [256, H]
            dn_pieces[p] = np.ascontiguousarray(
                d.reshape(2, 128, H).transpose(1, 0, 2)
            ).reshape(128, 2 * H)
        for q in range(-(-n_experts // 2)):
            parts = [dn_pieces[p] for p in (2 * q, 2 * q + 1) if p in dn_pieces]
            m[f"wdn{q}"] = np.ascontiguousarray(np.concatenate(parts, axis=1))
        in_maps.append(m)

    res = run_bass_kernel_spmd(nc, in_maps, core_ids=list(range(_NC)))
    global last_run
    last_run = res

    # ---- host-side reduction of the 8 partial outputs + unsort ----
    ysum = np.zeros((128, 8 * sum(chunk_ns)), np.float32)
    for s in range(_NC):
        ysum += np.asarray(res.results[s]["yT"]).astype(np.float32)
    out = np.zeros((T, H), np.float32)
    pos_tok = 0
    oc = 0
    for n in chunk_ns:
        blk = ysum[:, oc : oc + 8 * n].reshape(128, 8, n).transpose(2, 1, 0).reshape(n, H)
        out[order[pos_tok : pos_tok + n]] = blk
        pos_tok += n
        oc += 8 * n
    return out


# revision 25
# speedup vs baseline: 1.0013x; 1.0013x over previous
"""Tensor-parallel MoE MLP (Llama4 text experts) for 8 Trainium2 NeuronCores.

Strategy: 8-way tensor parallel over the intermediate dim F. Core s holds a
256-column slice of every expert's gate/up projection and the matching
256-row slice of its down projection (12 MB bf16 total, SBUF-resident).
Every core processes ALL 8192 tokens (sorted by expert id, chunked <=512
tokens per PSUM tile, chunk boundaries aligned to expert boundaries):
    h_s = up_s * silu(gate_s),   y_s = h_s @ Wd_s      (per-core partial)
    y   = sum_s y_s                                    (host-side add, free)
Compute per core is exactly total/8 independent of the expert distribution
(vs expert-parallel, which pays for the largest expert). Computed transposed
(weights stationary, tokens streaming); bf16 inputs, fp32 PSUM, bf16 partial
outputs (the host accumulates in fp32; adds ~0.4% rel err vs the 2e-2 gate).

Perf notes:
  - PE floor is 8192 tok x 48 cyc = 393216 cyc = 163.8us; expert-parallel's
    floor is max_count x 384 cyc (~173.4us for the graded seed).
  - First chunk is deliberately small (256 tokens) so the critical DMA set
    (pair-0 weights + half the x chunk, ~0.75 MB) lands early; warmup
    matmuls on zeroed SBUF lift the PE HAM clock gate meanwhile.
  - x chunks / weight pieces are released in consumption order, a few in
    flight at a time (GpSimd copy links gated on compute progress), so the
    SDMA packet round-robin cannot starve the critical transfers.
  - y partials stream out per chunk on the GpSimd (SWDGE) queue; the last
    chunk is small (128 tokens) and its output is split to shorten the tail.
"""

import numpy as np
import ml_dtypes

_BF16 = ml_dtypes.bfloat16
_NC = 8  # cores

_nc_cache: dict = {}
last_run = None  # BassKernelResults of the most recent kernel() call


def _equal_split(n, cap=512):
    if n <= 0:
        return []
    k = -(-n // cap)
    q, r = divmod(n, k)
    return [q + 1 if j < r else q for j in range(k)]


def _build(chunk_ns, chunk_epos, n_experts):
    import concourse.bacc as bacc
    import concourse.mybir as mybir
    from concourse.tile import TileContext

    nc = bacc.Bacc()
    bf16 = mybir.dt.bfloat16
    f32 = mybir.dt.float32

    KB1 = 8                      # H/128 contraction chunks for gate_up
    NCH = len(chunk_ns)
    n0 = chunk_ns[0]
    # column offset of each chunk in the packed x / y layouts
    xoff = [0]
    for n in chunk_ns:
        xoff.append(xoff[-1] + n)

    # DRAM inputs (host-packed, every DMA a whole-tensor contiguous copy)
    xc0a = nc.dram_tensor("xc0a", [128, 4 * n0], bf16, kind="ExternalInput")
    xc0b = nc.dram_tensor("xc0b", [128, 4 * n0], bf16, kind="ExternalInput")
    xcs = {
        i: nc.dram_tensor(f"xc{i}", [128, 8 * chunk_ns[i]], bf16, kind="ExternalInput")
        for i in range(1, NCH)
    }
    # expert 0's pair-0 weights split gate/up so the first-matmul critical
    # DMA is the 256 KB gate piece alone
    wgu0g = nc.dram_tensor("wgu0g", [128, 1024], bf16, kind="ExternalInput")
    wgu0u = nc.dram_tensor("wgu0u", [128, 1024], bf16, kind="ExternalInput")
    wgu0b = nc.dram_tensor("wgu0b", [128, 2048], bf16, kind="ExternalInput")
    wgus = {
        p: nc.dram_tensor(f"wgu{p}", [128, 4096], bf16, kind="ExternalInput")
        for p in range(1, n_experts)
    }
    n_wdn = -(-n_experts // 2)
    wdns = {
        q: nc.dram_tensor(
            f"wdn{q}", [128, 2048 * min(2, n_experts - 2 * q)], bf16,
            kind="ExternalInput",
        )
        for q in range(n_wdn)
    }
    yT = nc.dram_tensor("yT", [128, 8 * xoff[-1]], bf16, kind="ExternalOutput")

    # first chunk index of each expert position (for weight prefetch pacing)
    first_chunk_of_pos = {}
    for i, p in enumerate(chunk_epos):
        first_chunk_of_pos.setdefault(p, i)

    with TileContext(nc) as tc:
        with (
            tc.tile_pool(name="wpool", bufs=1) as wpool,
            tc.tile_pool(name="xpool", bufs=7) as xpool,
            tc.tile_pool(name="warm", bufs=1) as warm_p,
            tc.tile_pool(name="silu_p", bufs=3) as silu_p,
            tc.tile_pool(name="h_p", bufs=2) as h_p,
            tc.tile_pool(name="y_p", bufs=5) as y_p,
            tc.tile_pool(name="ps1", bufs=3, space="PSUM") as ps1_p,
            tc.tile_pool(name="ps2", bufs=5, space="PSUM") as ps2_p,
        ):
            # PE warm-up: dummy matmuls on zeroed SBUF while the critical
            # inputs stream in, lifting the HAM clock gate (1.2 -> 2.4 GHz)
            # before the real matmul stream.
            wtile = warm_p.tile([128, 256], bf16, name="wtile", tag="wtile")
            nc.vector.memset(wtile[:], 0.0)
            for i in range(44):
                ps_w = ps2_p.tile([128, 512], f32, tag="ps2")
                nc.tensor.matmul(
                    out=ps_w[:, :128],
                    lhsT=wtile[:, 128:256],
                    rhs=wtile[:, 0:128],
                    start=True,
                    stop=True,
                )

            # Resident weight tiles
            wgu_sb = wpool.tile([128, n_experts * 4096], bf16, name="wgu", tag="wgu")
            wdn_sb = wpool.tile([128, n_experts * 2048], bf16, name="wdn", tag="wdn")

            # Critical DMAs via SWDGE, strictly in first-use order: gate
            # pair-0 weights, then x chunk 0 (both halves), then up pair-0.
            # Nothing else competes for SDMA packets until these land.
            xt = {0: xpool.tile([128, 4096], bf16, name="xt0", tag="x")}
            nc.gpsimd.dma_start(out=wgu_sb[:, 0:1024], in_=wgu0g[:, :])
            nc.gpsimd.dma_start(out=xt[0][:, : 4 * n0], in_=xc0a[:, :])
            nc.gpsimd.dma_start(out=xt[0][:, 4 * n0 : 8 * n0], in_=xc0b[:, :])
            # pair-0 up / pair-1 weights on the sync ring in parallel with
            # the gpsimd critical set (the two rings share SDMA bandwidth
            # round-robin, so everything early streams concurrently)
            nc.sync.dma_start(out=wgu_sb[:, 1024:2048], in_=wgu0u[:, :])
            nc.sync.dma_start(out=wgu_sb[:, 2048:4096], in_=wgu0b[:, :])

            def release(dst_head, src_ap, engine, dst_ap, src_dram):
                # Gate a bulk DMA on earlier data/compute: GpSimd copy into
                # the head of the DMA's dest region, then the dma_start has
                # a WAW dependency on that copy.
                nc.gpsimd.tensor_copy(dst_head, src_ap)
                engine.dma_start(out=dst_ap, in_=src_dram)

            # chained early releases, gated on the first critical pieces
            wdn0_w = 2048 * min(2, n_experts)
            xc0a_tail = xt[0][:, 4 * n0 - 4 : 4 * n0]
            if NCH > 1:
                n1 = chunk_ns[1]
                xt[1] = xpool.tile([128, 4096], bf16, name="xt1", tag="x")
                release(
                    xt[1][:, :4], wgu_sb[:, 1020:1024],
                    nc.sync, xt[1][:, : 8 * n1], xcs[1][:, :],
                )
            release(
                wdn_sb[:, :4], xc0a_tail,
                nc.scalar, wdn_sb[:, :wdn0_w], wdns[0][:, :],
            )

            h_tiles = {}
            released_w = {0}
            released_dn = {0}

            def prefetch(i):
                # During chunk i's emission: release x chunk i+2 and any
                # weight pieces needed within the next ~2 chunks, gated on
                # chunk i-1's h tile (i.e. on compute progress).
                if i < 1:
                    return
                gate_src = h_tiles[(i - 1, 1)][:, :4]
                for j in range(i + 1, min(i + 4, NCH)):
                    if j not in xt:
                        nj = chunk_ns[j]
                        xt[j] = xpool.tile([128, 4096], bf16, name=f"xt{j}", tag="x")
                        release(
                            xt[j][:, :4], gate_src,
                            nc.sync, xt[j][:, : 8 * nj], xcs[j][:, :],
                        )
                # weight pieces for experts starting within ~5 chunks
                nw = 0
                for p in range(1, n_experts):
                    if p in released_w:
                        continue
                    if first_chunk_of_pos[p] <= i + 5:
                        release(
                            wgu_sb[:, p * 4096 : p * 4096 + 4], gate_src,
                            nc.scalar,
                            wgu_sb[:, p * 4096 : (p + 1) * 4096],
                            wgus[p][:, :],
                        )
                        released_w.add(p)
                        nw += 1
                        q = p // 2
                        if q not in released_dn:
                            o = q * 4096
                            w_q = 2048 * min(2, n_experts - 2 * q)
                            release(
                                wdn_sb[:, o : o + 4], gate_src,
                                nc.scalar,
                                wdn_sb[:, o : o + w_q],
                                wdns[q][:, :],
                            )
                            released_dn.add(q)
                        if nw >= 2:
                            break


            def gu_pair(i, p):
                # weight layout per (expert, pair): [gate k0..7 | up k0..7]
                e, n = chunk_epos[i], chunk_ns[i]
                ps_g = ps1_p.tile([128, 512], f32, tag="ps1")
                for k in range(KB1):
                    o = e * 4096 + p * 2048 + k * 128
                    nc.tensor.matmul(
                        out=ps_g[:, :n],
                        lhsT=wgu_sb[:, o : o + 128],
                        rhs=xt[i][:, k * n : k * n + n],
                        start=(k == 0),
                        stop=(k == KB1 - 1),
                    )
                ps_u = ps1_p.tile([128, 512], f32, tag="ps1")
                for k in range(KB1):
                    o = e * 4096 + p * 2048 + 1024 + k * 128
                    nc.tensor.matmul(
                        out=ps_u[:, :n],
                        lhsT=wgu_sb[:, o : o + 128],
                        rhs=xt[i][:, k * n : k * n + n],
                        start=(k == 0),
                        stop=(k == KB1 - 1),
                    )
                st = silu_p.tile([128, 512], bf16, tag="silu")
                nc.scalar.activation(
                    st[:, :n], ps_g[:, :n], mybir.ActivationFunctionType.Silu
                )
                ht = h_p.tile([128, 512], bf16, tag=f"h{p}")
                nc.vector.tensor_mul(out=ht[:, :n], in0=ps_u[:, :n], in1=st[:, :n])
                h_tiles[(i, p)] = ht

            def dn(i):
                e, n = chunk_epos[i], chunk_ns[i]
                last = i == NCH - 1
                yst = y_p.tile([128, 4096], bf16, tag="y")
                oc = 8 * xoff[i]
                for hh in range(8):
                    halves = [(0, n)]
                    if last and hh == 7:
                        halves = [(0, n // 2), (n // 2, n - n // 2)]
                    for c0, cn in halves:
                        ps_y = ps2_p.tile([128, 512], f32, tag="ps2")
                        for k2 in range(2):
                            nc.tensor.matmul(
                                out=ps_y[:, :cn],
                                lhsT=wdn_sb[:, e * 2048 + k2 * 1024 + hh * 128 : e * 2048 + k2 * 1024 + hh * 128 + 128],
                                rhs=h_tiles[(i, k2)][:, c0 : c0 + cn],
                                start=(k2 == 0),
                                stop=(k2 == 1),
                            )
                        # alternate PSUM->SBUF drains between DVE and ACT so
                        # neither engine's backlog stalls the dn matmul groups
                        dst = yst[:, hh * n + c0 : hh * n + c0 + cn]
                        if hh % 2 == 0:
                            nc.vector.tensor_copy(dst, ps_y[:, :cn])
                        else:
                            nc.scalar.activation(
                                dst, ps_y[:, :cn], mybir.ActivationFunctionType.Copy
                            )
                    if last and hh == 3:
                        nc.scalar.dma_start(
                            out=yT[:, oc : oc + 4 * n], in_=yst[:, : 4 * n]
                        )
                    if last and hh == 5:
                        nc.sync.dma_start(
                            out=yT[:, oc + 4 * n : oc + 6 * n],
                            in_=yst[:, 4 * n : 6 * n],
                        )
                    if last and hh == 7:
                        nc.scalar.dma_start(
                            out=yT[:, oc + 6 * n : oc + 7 * n + n // 2],
                            in_=yst[:, 6 * n : 7 * n + n // 2],
                        )
                del h_tiles[(i, 0)], h_tiles[(i, 1)]
                if last:
                    nc.sync.dma_start(
                        out=yT[:, oc + 7 * n + n // 2 : oc + 8 * n],
                        in_=yst[:, 7 * n + n // 2 : 8 * n],
                    )
                elif i >= NCH - 4:
                    # near the end, use the (now idle) sync HWDGE ring so
                    # output completion receipts don't delay the teardown
                    nc.sync.dma_start(out=yT[:, oc : oc + 8 * n], in_=yst[:, : 8 * n])
                else:
                    nc.gpsimd.dma_start(out=yT[:, oc : oc + 8 * n], in_=yst[:, : 8 * n])

            # Pipeline order: dn(i-1) is sandwiched BETWEEN chunk i's two
            # gu pairs. The PE bridge (gu pair) covers the silu+mul latency
            # of h(i-1), and the dn PSUM drains land in DVE/ACT FIFO order
            # right as the dn groups complete (no ps2 bank WAR exposure).
            for i in range(NCH):
                prefetch(i)
                gu_pair(i, 0)
                if i >= 1:
                    dn(i - 1)
                gu_pair(i, 1)
            dn(NCH - 1)
    nc.compile()
    return nc


def kernel(hidden_states, local_expert_indices, gate_up_proj, down_proj):
    from concourse.bass_utils import run_bass_kernel_spmd

    x = np.asarray(hidden_states, dtype=np.float32)
    idx = np.asarray(local_expert_indices).astype(np.int64)
    wgu_all = np.asarray(gate_up_proj, dtype=np.float32)
    wd_all = np.asarray(down_proj, dtype=np.float32)

    T, H = x.shape
    E, _, F2 = wgu_all.shape
    F = F2 // 2

    order = np.argsort(idx, kind="stable")
    counts = np.bincount(idx, minlength=E)
    experts = [e for e in range(E) if counts[e] > 0]
    n_experts = len(experts)

    # chunk plan: per expert, <=512-token chunks; small first chunk so the
    # critical first DMAs are small (fast ramp)
    chunk_ns, chunk_epos = [], []
    for pos, e in enumerate(experts):
        n = int(counts[e])
        if pos == 0 and n >= 320:
            sizes = [256] + _equal_split(n - 256)
        else:
            sizes = _equal_split(n)
        chunk_ns += sizes
        chunk_epos += [pos] * len(sizes)
    chunk_ns = tuple(chunk_ns)
    chunk_epos = tuple(chunk_epos)

    key = (H, F, chunk_ns, chunk_epos, n_experts)
    if key not in _nc_cache:
        _nc_cache[key] = _build(chunk_ns, chunk_epos, n_experts)
    nc = _nc_cache[key]

    # ---- pack shared x chunks (identical for every core) ----
    x_sorted = x[order].astype(_BF16)
    KB1 = H // 128
    shared = {}
    pos_tok = 0
    for i, n in enumerate(chunk_ns):
        blk = np.ascontiguousarray(
            x_sorted[pos_tok : pos_tok + n].reshape(n, KB1, 128).transpose(2, 1, 0)
        ).reshape(128, KB1 * n)
        if i == 0:
            shared["xc0a"] = np.ascontiguousarray(blk[:, : 4 * n])
            shared["xc0b"] = np.ascontiguousarray(blk[:, 4 * n :])
        else:
            shared[f"xc{i}"] = blk
        pos_tok += n

    # ---- per-core weight slices ----
    in_maps = []
    for s in range(_NC):
        sl = slice(s * 256, (s + 1) * 256)
        m = dict(shared)
        dn_pieces = {}
        for p, e in enumerate(experts):
            w = wgu_all[e]
            g = w[:, :F][:, sl].astype(_BF16)
            u = w[:, F:][:, sl].astype(_BF16)

            def kmaj(blk128):  # [H, 128] -> [128, KB1*128] (k-major lhsT)
                return np.ascontiguousarray(
                    blk128.reshape(KB1, 128, 128).transpose(1, 0, 2)
                ).reshape(128, KB1 * 128)

            # per pair: [gate k-major (1024) | up k-major (1024)]
            pair_blocks = [
                np.concatenate(
                    [kmaj(g[:, pp * 128 : (pp + 1) * 128]),
                     kmaj(u[:, pp * 128 : (pp + 1) * 128])],
                    axis=1,
                )
                for pp in range(2)
            ]
            if p == 0:
                m["wgu0g"] = np.ascontiguousarray(pair_blocks[0][:, :1024])
                m["wgu0u"] = np.ascontiguousarray(pair_blocks[0][:, 1024:])
                m["wgu0b"] = np.ascontiguousarray(pair_blocks[1])
            else:
                m[f"wgu{p}"] = np.ascontiguousarray(
                    np.concatenate(pair_blocks, axis=1)
                )
            d = wd_all[e][sl, :].astype(_BF16)  # [256, H]
            dn_pieces[p] = np.ascontiguousarray(
                d.reshape(2, 128, H).transpose(1, 0, 2)
            ).reshape(128, 2 * H)
        for q in range(-(-n_experts // 2)):
            parts = [dn_pieces[p] for p in (2 * q, 2 * q + 1) if p in dn_pieces]
            m[f"wdn{q}"] = np.ascontiguousarray(np.concatenate(parts, axis=1))
        in_maps.append(m)

    res = run_bass_kernel_spmd(nc, in_maps, core_ids=list(range(_NC)))
    global last_run
    last_run = res

    # ---- host-side reduction of the 8 partial outputs + unsort ----
    ysum = np.zeros((128, 8 * sum(chunk_ns)), np.float32)
    for s in range(_NC):
        ysum += np.asarray(res.results[s]["yT"]).astype(np.float32)
    out = np.zeros((T, H), np.float32)
    pos_tok = 0
    oc = 0
    for n in chunk_ns:
        blk = ysum[:, oc : oc + 8 * n].reshape(128, 8, n).transpose(2, 1, 0).reshape(n, H)
        out[order[pos_tok : pos_tok + n]] = blk
        pos_tok += n
        oc += 8 * n
    return out
